# revision 1
# baseline (speedup 1.0000x reference)
"""Multi-head causal self-attention on 8 Trainium2 NeuronCores.

Tensor-parallel over heads: core i owns heads (2i, 2i+1).
Per core:
  phase 1: qT/kT/vT = (W_slice^T @ x^T) for its 2 heads (xT provided by host);
           vT transposed on PE into [token, d] tiles for both heads at once.
  phase 2: per (b, h), k-chunk-outer loop sharing each stationary operand
           across all valid q-blocks: scoresT[k,q] = K Q^T -> exp ->
           (causal mask) -> out[d+1, q] accumulated as [V | 1]^T @ attnT
           (extra row = softmax denominator); normalize via 1/l broadcast.
  phase 3: partial projection P_i = W_proj[own rows]^T @ A_i, chunked over
           token ranges; ReduceScatter(add) each chunk across the 8 cores
           (overlaps with remaining attention compute); + bias.
Host reassembles the 8 column slices.

Matmuls run as float32r (fp32 storage, fast PE mode): ~230 ns per 512-col
matmul vs 429 ns for fp32, at ~1e-4 component relative error.
"""

import os

import numpy as np

B, T, C, H = 2, 2048, 1024, 16
D = C // H            # 64
NCORES = 8
HL = H // NCORES      # 2 heads per core
NT = B * T            # 4096
NQ = T // 512         # q-blocks of 512 per (b,h)
NK = T // 128         # k-chunks of 128 per (b,h)
SCALE = float(D) ** -0.5

MM_FAST = True        # float32r matmuls vs float32

_cache = {}


def _build(mode: str):
    """mode: 'causal' | 'none' (all-ones mask)."""
    import concourse.mybir as mybir
    import concourse.tile as tile
    from concourse import bacc

    f32 = mybir.dt.float32
    mdt = mybir.dt.float32r if MM_FAST else f32

    nc = bacc.Bacc("TRN2", target_bir_lowering=False, debug=False,
                   num_devices=NCORES)
    xT = nc.dram_tensor("xT", [C, NT], mdt, kind="ExternalInput").ap()
    wqkv = nc.dram_tensor("wqkv", [C, 3 * HL * D], mdt,
                          kind="ExternalInput").ap()
    wp = nc.dram_tensor("wp", [C, 128], mdt, kind="ExternalInput").ap()
    bias = nc.dram_tensor("bias", [128, 1], f32, kind="ExternalInput").ap()
    cmask = nc.dram_tensor("cmask", [128, 4 * 512 + 128], f32,
                           kind="ExternalInput").ap()
    ones_in = nc.dram_tensor("ones_in", [128, 64], mdt,
                             kind="ExternalInput").ap()
    outT = nc.dram_tensor("outT", [128, NT], f32, kind="ExternalOutput").ap()

    causal = mode == "causal"
    Exp = mybir.ActivationFunctionType.Exp

    with tile.TileContext(nc) as tc, \
         nc.allow_low_precision(reason="float32r matmul fast path"):
        with tc.tile_pool(name="persist", bufs=1) as persist, \
             tc.tile_pool(name="dram", bufs=1, space="DRAM") as dram:
            q_sb = persist.tile([128, NT], mdt)
            k_sb = persist.tile([128, NT], mdt)
            # V tiles, both heads, each with a trailing ones column:
            # cols 0:64 = head0 d, col 64 = ones, 65:129 = head1 d, 129 = ones
            vboth = persist.tile([128, 131, B * NK], mdt)
            a_sb = persist.tile([128, NT], mdt)   # normalized attn out
            cm_sb = persist.tile([128, 4 * 512 + 128], f32)
            ones_sb = persist.tile([1, 64], mdt)
            wqkv_sb = persist.tile([128, 8, 3 * HL * D], mdt)
            wp_sb = persist.tile([128, 8, 128], mdt)
            bias_sb = persist.tile([128, 1], f32)
            ag_in0 = dram.tile([128, T], mdt)
            ag_in1 = dram.tile([128, T], mdt)
            ag_out0 = dram.tile([C, T], mdt, addr_space="Shared")
            ag_out1 = dram.tile([C, T], mdt, addr_space="Shared")
            ag_ins = [ag_in0, ag_in1]
            ag_outs = [ag_out0, ag_out1]

            nc.sync.dma_start(out=cm_sb[:], in_=cmask[:])
            nc.sync.dma_start(out=wqkv_sb[:],
                              in_=wqkv.rearrange("(a p) n -> p a n", p=128))
            nc.sync.dma_start(out=wp_sb[:],
                              in_=wp.rearrange("(a p) n -> p a n", p=128))
            nc.sync.dma_start(out=bias_sb[:], in_=bias[:])
            nc.sync.dma_start(out=ones_sb[:], in_=ones_in[0:1, :])
            nc.sync.dma_start(out=vboth[:, 64, :], in_=ones_in[:, 0:B * NK])
            nc.sync.dma_start(out=vboth[:, 130 - 1, :],
                              in_=ones_in[:, 0:B * NK])
            ident = cm_sb[:, 4 * 512:4 * 512 + 128]

            # ---- phase 1: qkvT = W_slice^T @ xT ----
            with tc.tile_pool(name="xn_pool", bufs=6) as xp, \
                 tc.tile_pool(name="qkv_psum", bufs=4, space="PSUM") as pp, \
                 tc.tile_pool(name="vt_psum", bufs=2, space="PSUM") as vtp, \
                 tc.tile_pool(name="vtmp_pool", bufs=2) as vpool:
                for ng in range(2):
                    xns = []
                    for k in range(4):
                        n = ng * 4 + k
                        xn = xp.tile([128, 8, 512], mdt, tag="xn")
                        nc.sync.dma_start(
                            out=xn[:],
                            in_=xT.rearrange("(a p) t -> p a t", p=128)
                            [:, :, n * 512:(n + 1) * 512])
                        xns.append((n, xn))
                    for m in range(3):  # 0: q, 1: k, 2: v
                        pss = [pp.tile([128, 512], f32, tag="qkv", name="qkvp")
                               for _ in range(4)]
                        for kc in range(8):
                            for idx, (n, xn) in enumerate(xns):
                                nc.tensor.matmul(
                                    pss[idx][:],
                                    wqkv_sb[:, kc, m * 128:(m + 1) * 128],
                                    xn[:, kc, :],
                                    start=(kc == 0), stop=(kc == 7))
                        for idx, (n, xn) in enumerate(xns):
                            ps = pss[idx]
                            tok = slice(n * 512, (n + 1) * 512)
                            if m == 0:
                                nc.vector.tensor_copy(q_sb[:, tok], ps[:])
                            elif m == 1:
                                nc.vector.tensor_copy(k_sb[:, tok], ps[:])
                            else:
                                vtmp = vpool.tile([128, 512], f32)
                                nc.vector.tensor_copy(vtmp[:], ps[:])
                                b = n // NQ
                                for s in range(4):
                                    j = b * NK + (n % NQ) * 4 + s
                                    pt = vtp.tile([128, 128], f32)
                                    nc.tensor.transpose(
                                        pt[:],
                                        vtmp[:, s * 128:(s + 1) * 128],
                                        ident)
                                    nc.vector.tensor_copy(
                                        vboth[:, 0:64, j], pt[:, 0:64])
                                    nc.vector.tensor_copy(
                                        vboth[:, 65:129, j], pt[:, 64:128])

            # ---- phase 2 + 3: attention, then per-b proj + ReduceScatter ----
            with tc.tile_pool(name="s_psum", bufs=3, space="PSUM") as sp, \
                 tc.tile_pool(name="o_psum", bufs=4, space="PSUM") as op, \
                 tc.tile_pool(name="p_psum", bufs=1, space="PSUM") as prp, \
                 tc.tile_pool(name="attn_pool", bufs=6) as apool, \
                 tc.tile_pool(name="small_pool", bufs=2) as smp, \
                 tc.tile_pool(name="rb_pool", bufs=2) as rbp, \
                 tc.tile_pool(name="out_pool", bufs=2) as outp:
                for b in range(B):
                    for h in range(HL):
                        hs = slice(h * 64, (h + 1) * 64)
                        vcols = slice(65 * h, 65 * h + 65)
                        lrow = 64
                        drows = slice(0, 64)
                        po = [op.tile([65, 512], f32, tag="po", name="po")
                              for _ in range(NQ)]
                        for ki in range(NK):
                            qj0 = ki // 4 if causal else 0
                            ats = {}
                            for qj in range(qj0, NQ):
                                ps = sp.tile([128, 512], f32, tag="s")
                                nc.tensor.matmul(
                                    ps[:],
                                    k_sb[hs, b * T + ki * 128:
                                         b * T + (ki + 1) * 128],
                                    q_sb[hs, b * T + qj * 512:
                                         b * T + (qj + 1) * 512],
                                    start=True, stop=True)
                                at = apool.tile([128, 512], mdt, tag="at")
                                nc.scalar.activation(at[:], ps[:], Exp,
                                                     scale=SCALE)
                                if causal and qj == qj0:
                                    nc.vector.tensor_mul(
                                        at[:], at[:],
                                        cm_sb[:, (ki % 4) * 512:
                                              (ki % 4 + 1) * 512])
                                ats[qj] = at
                            for qj in range(qj0, NQ):
                                last = (4 * qj + 3) if causal else (NK - 1)
                                nc.tensor.matmul(
                                    po[qj][:], vboth[:, vcols, b * NK + ki],
                                    ats[qj][:],
                                    start=(ki == 0), stop=(ki == last))
                        # normalize: A = po[d] * (1/l) (l broadcast via PE)
                        lsb = smp.tile([1, 2048], f32)
                        for qj in range(NQ):
                            nc.vector.tensor_copy(
                                lsb[0:1, qj * 512:(qj + 1) * 512],
                                po[qj][lrow:lrow + 1, :])
                        ras = smp.tile([1, 2048], f32)
                        nc.vector.reciprocal_approx_fast(ras[:], lsb[:])
                        rl = smp.tile([1, 2048], mdt)
                        nc.vector.tensor_copy(rl[:], ras[:])
                        rb = rbp.tile([64, 2048], f32)
                        for qj in range(NQ):
                            pb = sp.tile([64, 512], f32, tag="s", name="pb")
                            nc.tensor.matmul(
                                pb[:], ones_sb[:],
                                rl[0:1, qj * 512:(qj + 1) * 512],
                                start=True, stop=True)
                            nc.vector.tensor_copy(
                                rb[:, qj * 512:(qj + 1) * 512], pb[:])
                        for qj in range(NQ):
                            tok = slice(b * T + qj * 512,
                                        b * T + (qj + 1) * 512)
                            nc.vector.tensor_mul(
                                a_sb[hs, tok], po[qj][drows, :],
                                rb[:, qj * 512:(qj + 1) * 512])
                    # stage this b's A slice for the AllGather
                    nc.sync.dma_start(out=ag_ins[b][:],
                                      in_=a_sb[:, b * T:(b + 1) * T])
                # ---- AllGather + local proj, per b (after all attention) ----
                for b in range(B):
                    t0 = b * T
                    nc.gpsimd.collective_compute(
                        "AllGather", mybir.AluOpType.bypass,
                        replica_groups=[list(range(NCORES))],
                        ins=[ag_ins[b].opt()], outs=[ag_outs[b].opt()])
                    for nn2 in range(4):
                        agt = outp.tile([128, 8, 512], mdt, tag="agt")
                        nc.sync.dma_start(
                            out=agt[:],
                            in_=ag_outs[b].rearrange("(a p) t -> p a t", p=128)
                            [:, :, nn2 * 512:(nn2 + 1) * 512])
                        pr = prp.tile([128, 512], f32, tag="pr")
                        for kc in range(8):
                            nc.tensor.matmul(
                                pr[:], wp_sb[:, kc, :], agt[:, kc, :],
                                start=(kc == 0), stop=(kc == 7))
                        ot = outp.tile([128, 512], f32, tag="ot")
                        nc.vector.tensor_scalar_add(ot[:], pr[:], bias_sb[:])
                        nc.sync.dma_start(
                            out=outT[:, t0 + nn2 * 512:t0 + (nn2 + 1) * 512],
                            in_=ot[:])

    nc.compile()
    return nc


def _get_program(mode: str):
    if mode not in _cache:
        _cache[mode] = _build(mode)
    return _cache[mode]


def kernel(**inputs):
    from concourse.bass_utils import run_bass_kernel_spmd

    x = np.ascontiguousarray(np.asarray(inputs["x"], dtype=np.float32))
    mask = np.asarray(inputs["causal_mask"])
    Wqkv = np.ascontiguousarray(np.asarray(inputs["W_qkv"], dtype=np.float32))
    Wp = np.ascontiguousarray(np.asarray(inputs["W_proj"], dtype=np.float32))
    bp = np.asarray(inputs["b_proj"], dtype=np.float32)

    m2 = np.asarray(mask).reshape(T, T)
    if np.all(m2 != 0):
        mode = "none"
    else:
        tril = np.tril(np.ones((T, T), dtype=m2.dtype))
        if np.array_equal(m2, tril):
            mode = "causal"
        else:
            raise NotImplementedError("general mask not supported")

    nc = _get_program(mode)

    xT = np.ascontiguousarray(x.reshape(NT, C).T)  # [C, NT]

    # causal-mask tile patterns (valid iff p <= f - 128*j) + 128x128 identity
    p = np.arange(128)[:, None]
    f = np.arange(512)[None, :]
    cm = np.concatenate(
        [(p <= f - 128 * j).astype(np.float32) for j in range(4)]
        + [np.eye(128, dtype=np.float32)], axis=1)

    Wq = Wqkv[:, 0 * C:1 * C]
    Wk = Wqkv[:, 1 * C:2 * C]
    Wv = Wqkv[:, 2 * C:3 * C]

    in_maps = []
    for i in range(NCORES):
        hcols = slice(2 * i * D, (2 * i + 2) * D)  # this core's 2 heads
        wqkv_i = np.concatenate(
            [Wq[:, hcols], Wk[:, hcols], Wv[:, hcols]], axis=1)  # [C, 384]
        in_maps.append({
            "xT": xT,
            "wqkv": np.ascontiguousarray(wqkv_i),
            "wp": np.ascontiguousarray(Wp[:, i * 128:(i + 1) * 128]),
            "bias": np.ascontiguousarray(bp[i * 128:(i + 1) * 128]
                                         .reshape(128, 1)),
            "cmask": cm,
            "ones_in": np.ones((128, 64), dtype=np.float32),
        })

    res = run_bass_kernel_spmd(nc, in_maps, list(range(NCORES)))

    out = np.empty((NT, C), dtype=np.float32)
    for i in range(NCORES):
        out[:, i * 128:(i + 1) * 128] = res.results[i]["outT"].T
    return out.reshape(B, T, C)



# revision 15
# speedup vs baseline: 1.4410x; 1.4410x over previous
"""Multi-head causal self-attention on 8 Trainium2 NeuronCores.

Tensor-parallel over heads: core i owns heads (2i, 2i+1).

Per core:
  phase 1: qkvT = (W_slice^T @ x^T) for its 2 heads, with x and W_qkv in
           fp8-e4m3 and DoubleRow matmuls (two 128-row contraction chunks
           per pass, 2x throughput). W_qkv is pre-scaled by 16 on the host
           to dodge the fp8 subnormal range; the PSUM->SBUF cast divides
           it back. q/k land in fp16; v is transposed on the PE into
           [token, head, d] fp8 tiles.
  phase 2: per (b, h), ki outer loop: scoresT[k, q] = K Q^T (fp16),
           batched 2 q-blocks per PSUM tile so exp runs as up-to-[128,1024]
           Act instructions (fully-masked column spans are trimmed from
           both the scores matmul and the exp). attn tiles are fp16;
           attn@V uses a 65-wide [V | 1] stationary so the softmax
           denominator accumulates as po row 64 for free. Diagonal tiles:
           the masked prefix of the attn tile is zeroed (DVE memset) and
           the 128-wide triangle masked with one small multiply.
           Normalization broadcasts 1/l over 64 rows via an f32r matmul.
  phase 3: per b: the A slice is staged and AllGather'd (fp16) right
           after that b's attention, so AG(b0) overlaps b1's attention.
           Projection readback rides the sync DMA queue (collectives own
           the gpsimd queue); proj(b0) is emitted between b1's two heads
           so its PSUM comes from the just-released po ring slots and its
           matmuls never stall the attention PE stream.
Host reassembles the 8 column slices.
"""

import numpy as np

B, T, C, H = 2, 2048, 1024, 16
D = C // H            # 64
NCORES = 8
HL = H // NCORES      # 2 heads per core
NT = B * T            # 4096
NQ = T // 512         # 4 q-blocks per (b,h)
NK = T // 128         # 16 k-chunks per (b,h)
NKP = NK // 2         # 8 k-chunk pairs
SCALE = float(D) ** -0.5
WSCALE = 16.0         # host multiplies W_qkv by this; kernel divides back
PH1_FP8 = False       # fp8 DoubleRow for the QKV projection

_cache = {}


def _build(mode: str):
    """mode: 'causal' | 'none' (all-ones mask)."""
    import concourse.mybir as mybir
    import concourse.tile as tile
    from concourse import bacc

    f32 = mybir.dt.float32
    f16 = mybir.dt.float16
    f8 = mybir.dt.float8e4
    p1dt = f8 if PH1_FP8 else f16
    DR = mybir.MatmulPerfMode.DoubleRow

    causal = mode == "causal"
    Exp = mybir.ActivationFunctionType.Exp
    Copy = mybir.ActivationFunctionType.Copy

    nc = bacc.Bacc("TRN2", target_bir_lowering=False, debug=False,
                   num_devices=NCORES)
    xT = nc.dram_tensor("xT", [C, NT], p1dt, kind="ExternalInput").ap()
    wqkv = nc.dram_tensor("wqkv", [C, 3 * HL * D], p1dt,
                          kind="ExternalInput").ap()
    wp = nc.dram_tensor("wp", [C, 128], f16, kind="ExternalInput").ap()
    bias = nc.dram_tensor("bias", [128, 1], f32, kind="ExternalInput").ap()
    ident_in = nc.dram_tensor("ident", [128, 128], f16,
                              kind="ExternalInput").ap()
    tri_in = nc.dram_tensor("tri", [128, 128], f16,
                            kind="ExternalInput").ap()
    outT = nc.dram_tensor("outT", [128, NT], f32, kind="ExternalOutput").ap()

    with tile.TileContext(nc) as tc, \
         nc.allow_low_precision(reason="fp8/fp16 matmul fast path"):
        with tc.tile_pool(name="persist", bufs=1) as persist, \
             tc.tile_pool(name="dram", bufs=1, space="DRAM") as dram:
            q_sb = persist.tile([128, NT], f16)
            k_sb = persist.tile([128, NT], f16)
            # V in [token, chunk, head, d|1] fp16 layout; col 64 = ones
            vboth = persist.tile([128, B * NK, HL, D + 1], f16)
            a_sb = [persist.tile([128, T], f16, name=f"a_sb{i}")
                    for i in range(B)]
            ident = persist.tile([128, 128], f16)
            tri = persist.tile([128, 128], f16)
            onesb = persist.tile([128, D], f32)
            wqkv_sb = persist.tile([128, 8, 3 * HL * D], p1dt)
            wp_sb = persist.tile([128, 8, 128], f16)
            bias_sb = persist.tile([128, 1], f32)
            ag_ins = [dram.tile([128, T], f16, name=f"ag_in{i}")
                      for i in range(B)]
            ag_outs = [dram.tile([C, T], f16, addr_space="Shared",
                                 name=f"ag_out{i}") for i in range(B)]

            # weights first on the sync queue (the xn stream follows it)
            nc.sync.dma_start(out=wqkv_sb[:],
                              in_=wqkv.rearrange("(a p) n -> p a n", p=128))
            # non-urgent loads go on the gpsimd queue
            nc.gpsimd.dma_start(out=ident[:], in_=ident_in[:])
            nc.gpsimd.dma_start(out=tri[:], in_=tri_in[:])
            nc.gpsimd.dma_start(out=wp_sb[:],
                                in_=wp.rearrange("(a p) n -> p a n", p=128))
            nc.gpsimd.dma_start(out=bias_sb[:], in_=bias[:])
            nc.vector.memset(onesb[:], 1.0)
            nc.vector.memset(vboth[:, :, :, D], 1.0)

            # ---- phase 1: qkvT = W_slice^T @ xT ----
            with tc.tile_pool(name="xn_pool", bufs=6) as xp, \
                 tc.tile_pool(name="qkv_psum", bufs=4, space="PSUM") as pp, \
                 tc.tile_pool(name="vt_psum", bufs=2, space="PSUM") as vtp, \
                 tc.tile_pool(name="vtmp_pool", bufs=2) as vpool:
                for ng in range(2):
                    xns = []
                    for k in range(4):
                        n = ng * 4 + k
                        xn = xp.tile([128, 8, 512], p1dt, tag="xn")
                        nc.sync.dma_start(
                            out=xn[:],
                            in_=xT.rearrange("(a p) t -> p a t", p=128)
                            [:, :, n * 512:(n + 1) * 512])
                        xns.append((n, xn))
                    for m in range(3):  # 0: q, 1: k, 2: v
                        pss = [pp.tile([128, 512], f32, tag="qkv",
                                       name="qkvp") for _ in range(4)]
                        if PH1_FP8:
                            for kcp in range(4):
                                for idx, (n, xn) in enumerate(xns):
                                    nc.tensor.matmul(
                                        pss[idx][:],
                                        wqkv_sb[:, 2 * kcp:2 * kcp + 2,
                                                m * 128:(m + 1) * 128],
                                        xn[:, 2 * kcp:2 * kcp + 2, :],
                                        start=(kcp == 0), stop=(kcp == 3),
                                        perf_mode=DR)
                        else:
                            for kc in range(8):
                                for idx, (n, xn) in enumerate(xns):
                                    nc.tensor.matmul(
                                        pss[idx][:],
                                        wqkv_sb[:, kc, m * 128:(m + 1) * 128],
                                        xn[:, kc, :],
                                        start=(kc == 0), stop=(kc == 7))
                        for idx, (n, xn) in enumerate(xns):
                            ps = pss[idx]
                            tok = slice(n * 512, (n + 1) * 512)
                            if m == 0:
                                nc.scalar.activation(q_sb[:, tok], ps[:],
                                                     Copy, scale=1.0 / WSCALE)
                            elif m == 1:
                                nc.scalar.activation(k_sb[:, tok], ps[:],
                                                     Copy, scale=1.0 / WSCALE)
                            else:
                                vtmp = vpool.tile([128, 512], f16)
                                nc.vector.tensor_scalar_mul(
                                    vtmp[:], ps[:], 1.0 / WSCALE)
                                b = n // NQ
                                for s in range(4):
                                    j = b * NK + (n % NQ) * 4 + s
                                    pt = vtp.tile([128, 128], f16)
                                    nc.tensor.transpose(
                                        pt[:],
                                        vtmp[:, s * 128:(s + 1) * 128],
                                        ident[:])
                                    # alternate engines for the copy-out
                                    if s % 2 == 0:
                                        nc.vector.tensor_copy(
                                            vboth[:, j, :, 0:D], pt[:])
                                    else:
                                        nc.scalar.activation(
                                            vboth[:, j, :, 0:D], pt[:], Copy)

            # ---- phase 2 + 3: attention; per-b AllGather right after ----
            def proj(b):
                t0 = b * T
                for nn2 in range(4):
                    agt = outp.tile([128, 8, 512], f16, tag="agt")
                    nc.sync.dma_start(
                        out=agt[:],
                        in_=ag_outs[b].rearrange("(a p) t -> p a t", p=128)
                        [:, :, nn2 * 512:(nn2 + 1) * 512])
                    pr = op.tile([128, 512], f32, tag="po", name="pr")
                    for kc in range(8):
                        nc.tensor.matmul(
                            pr[:], wp_sb[:, kc, :], agt[:, kc, :],
                            start=(kc == 0), stop=(kc == 7))
                    ot = outp.tile([128, 512], f32, tag="ot")
                    nc.vector.tensor_scalar_add(ot[:], pr[:], bias_sb[:])
                    nc.scalar.dma_start(
                        out=outT[:, t0 + nn2 * 512:t0 + (nn2 + 1) * 512],
                        in_=ot[:])

            with tc.tile_pool(name="s_psum", bufs=2, space="PSUM") as sp, \
                 tc.tile_pool(name="o_psum", bufs=4, space="PSUM") as op, \
                 tc.tile_pool(name="attn_pool", bufs=4) as apool, \
                 tc.tile_pool(name="small_pool", bufs=2) as smp, \
                 tc.tile_pool(name="rb_pool", bufs=2) as rbp, \
                 tc.tile_pool(name="out_pool", bufs=4) as outp:
                for b in range(B):
                    for h in range(HL):
                        hs = slice(h * D, (h + 1) * D)
                        po = [op.tile([65, 512], f32, tag="po", name="po")
                              for _ in range(NQ)]
                        for ki in range(NK):
                            qj0 = ki // 4 if causal else 0
                            off = (ki % 4) * 128
                            krng = slice(b * T + ki * 128,
                                         b * T + (ki + 1) * 128)
                            chunks = []
                            qjc = qj0
                            while qjc < NQ:
                                chunks.append((qjc, min(2, NQ - qjc)))
                                qjc += 2
                            for (qjc, w) in chunks:
                                diag = causal and qjc == qj0
                                lo = off if diag else 0
                                ps = sp.tile([128, 1024], f32, tag="s")
                                for j2 in range(w):
                                    qj = qjc + j2
                                    cl = lo if j2 == 0 else 0
                                    nc.tensor.matmul(
                                        ps[:, j2 * 512 + cl:(j2 + 1) * 512],
                                        k_sb[hs, krng],
                                        q_sb[hs, b * T + qj * 512 + cl:
                                             b * T + (qj + 1) * 512],
                                        start=True, stop=True)
                                at = apool.tile([128, 1024], f16, tag="at")
                                nc.scalar.activation(
                                    at[:, lo:512 * w], ps[:, lo:512 * w],
                                    Exp, scale=SCALE)
                                if diag:
                                    if lo > 0:
                                        nc.vector.memset(at[:, 0:lo], 0.0)
                                    nc.vector.tensor_mul(
                                        at[:, lo:lo + 128],
                                        at[:, lo:lo + 128], tri[:])
                                for j2 in range(w):
                                    qj = qjc + j2
                                    last = (4 * qj + 3) if causal else NK - 1
                                    nc.tensor.matmul(
                                        po[qj][:],
                                        vboth[:, b * NK + ki, h, :],
                                        at[:, j2 * 512:(j2 + 1) * 512],
                                        start=(ki == 0), stop=(ki == last))
                        # normalize: A = po * (1/l) (l broadcast via PE)
                        lsb = smp.tile([1, 2048], f32, name="lsb")
                        for qj in range(NQ):
                            nc.vector.tensor_copy(
                                lsb[0:1, qj * 512:(qj + 1) * 512],
                                po[qj][D:D + 1, :])
                        ras = smp.tile([1, 2048], f32, name="ras")
                        for qj in range(NQ):
                            nc.vector.reciprocal_approx_fast(
                                ras[0:1, qj * 512:(qj + 1) * 512],
                                lsb[0:1, qj * 512:(qj + 1) * 512])
                        rb = rbp.tile([64, 2048], f32)
                        for qj in range(NQ):
                            pb = sp.tile([64, 512], f32, tag="s", name="pb")
                            nc.tensor.matmul(
                                pb[:], onesb[0:1, :],
                                ras[0:1, qj * 512:(qj + 1) * 512],
                                start=True, stop=True)
                            nc.vector.tensor_copy(
                                rb[:, qj * 512:(qj + 1) * 512], pb[:])
                        for qj in range(NQ):
                            nc.vector.tensor_mul(
                                a_sb[b][hs, qj * 512:(qj + 1) * 512],
                                po[qj][0:D, :],
                                rb[:, qj * 512:(qj + 1) * 512])
                        if b == 1 and h == 0:
                            # proj(b0): AG0 done by now; PSUM comes from
                            # po slots this head just released
                            proj(0)
                    # stage + AllGather this b immediately (overlaps the
                    # next b's attention)
                    nc.gpsimd.dma_start(out=ag_ins[b][:], in_=a_sb[b][:])
                    nc.gpsimd.collective_compute(
                        "AllGather", mybir.AluOpType.bypass,
                        replica_groups=[list(range(NCORES))],
                        ins=[ag_ins[b].opt()], outs=[ag_outs[b].opt()])
                # ---- projection for b=1: the kernel tail ----
                proj(1)

    nc.compile()
    return nc


def _get_program(mode: str):
    if mode not in _cache:
        _cache[mode] = _build(mode)
    return _cache[mode]


def kernel(**inputs):
    import ml_dtypes
    from concourse.bass_utils import run_bass_kernel_spmd

    f8np = ml_dtypes.float8_e4m3
    p1np = f8np if PH1_FP8 else np.float16

    x = np.ascontiguousarray(np.asarray(inputs["x"], dtype=np.float32))
    mask = np.asarray(inputs["causal_mask"])
    Wqkv = np.ascontiguousarray(np.asarray(inputs["W_qkv"], dtype=np.float32))
    Wp = np.ascontiguousarray(np.asarray(inputs["W_proj"], dtype=np.float32))
    bp = np.asarray(inputs["b_proj"], dtype=np.float32)

    m2 = np.asarray(mask).reshape(T, T)
    if np.all(m2 != 0):
        mode = "none"
    else:
        tril = np.tril(np.ones((T, T), dtype=m2.dtype))
        if np.array_equal(m2, tril):
            mode = "causal"
        else:
            raise NotImplementedError("general mask not supported")

    nc = _get_program(mode)

    xT = np.ascontiguousarray(x.reshape(NT, C).T).astype(p1np)  # [C, NT]

    # 128x128 lower-triangle (keep k <= q within the diagonal block)
    p = np.arange(128)[:, None]
    f = np.arange(128)[None, :]
    tri = (p <= f).astype(np.float16)
    ident = np.eye(128, dtype=np.float16)

    Wq = Wqkv[:, 0 * C:1 * C]
    Wk = Wqkv[:, 1 * C:2 * C]
    Wv = Wqkv[:, 2 * C:3 * C]

    in_maps = []
    for i in range(NCORES):
        hcols = slice(2 * i * D, (2 * i + 2) * D)  # this core's 2 heads
        wqkv_i = np.concatenate(
            [Wq[:, hcols], Wk[:, hcols], Wv[:, hcols]], axis=1)  # [C, 384]
        in_maps.append({
            "xT": xT,
            "wqkv": (wqkv_i * WSCALE).astype(p1np),
            "wp": np.ascontiguousarray(
                Wp[:, i * 128:(i + 1) * 128]).astype(np.float16),
            "bias": np.ascontiguousarray(bp[i * 128:(i + 1) * 128]
                                         .reshape(128, 1)),
            "ident": ident,
            "tri": tri,
        })

    res = run_bass_kernel_spmd(nc, in_maps, list(range(NCORES)))

    out = np.empty((NT, C), dtype=np.float32)
    for i in range(NCORES):
        out[:, i * 128:(i + 1) * 128] = res.results[i]["outT"].T
    return out.reshape(B, T, C)
